# revision 25
# baseline (speedup 1.0000x reference)
"""BitConv2d forward on 8 Trainium2 NeuronCores (SPMD data-parallel).

Strategy (v9 -- even/odd row-parity K-packing):
  - Shard batch (32) -> 4 images per core; replicate the tiny bit-plane
    weights/scales on every core. No collectives needed (forward only).
  - x and y move through HBM as bf16 AND in row-parity-plane layout
    [B, C, 2, 56, W] (host numpy pre/post shuffles -- pure data layout prep,
    no conv math on the host). Precision ~4e-3 max rel err vs the 2e-2 gate.
  - The parity layout packs the PE contraction dim: partitions 0:64 hold the
    EVEN padded rows of the image (cin-major), partitions 64:128 the ODD
    padded rows. One 128x128 stationary operand then carries TWO vertical
    taps for BOTH output-row parities (3 of its 4 64x64 blocks non-zero), so
    the 3x3 conv needs 6 accumulating matmuls per output tile instead of 9:
       s=0,u: [[Wt(0,u), 0], [Wt(1,u), Wt(0,u)]]
       s=1,u: [[Wt(2,u), Wt(1,u)], [0, Wt(2,u)]]   (K-blocks x M-parities)
    75% PE utilization vs 50% for the classic block-diagonal halves scheme.
  - NO column padding: rows are stored 112-contiguous, horizontal taps wrap
    across row boundaries, and the wrap contributions are cancelled exactly
    by 4 small fixup matmuls per image (N=56 stride-112 views, reusing the
    same stationary tiles) subtracted at output cols 0 and 111.
  - Every HBM<->SBUF transfer is large contiguous descriptors; all DMA on
    HWDGE (input on the sync ring, output on the scalar ring). PSUM tiles
    N=448 = 4 row-pairs = 8 output rows; epilogue (scale+bias, f32 psum ->
    bf16) is a single contiguous DVE op per tile; output streams out behind
    the epilogue in row-pair groups.
  - Dummy matmuls at kernel start keep the PE HAM-warm through the weight
    load so the first real tile runs at 2.4 GHz.
"""

import numpy as np

B, C, H, W = 32, 64, 112, 112
NB = 4
CORES = 8
BPC = B // CORES  # images per core

HALF = H // 2  # 56 row-pairs (and 56 rows per output plane)
D = 1  # data base column (one zero col in front)
NROW0 = 57  # block rows incl the zero pad row
XC = D + NROW0 * W + 115  # 6500 total cols
OUTC = HALF * W  # 6272 output cols per partition (one parity plane)

NT = 448  # = 4*112: one PSUM tile covers 4 row-pairs = 8 output rows
NTILES = 14  # 14*448 = 6272
XBUFS = 3

# input chunks in row-pair units (conv tile t needs block rows <= 4t+4)
IN_CHUNKS = [(0, 19), (19, 38), (38, 56)]

_CACHE = {}


def _build():
    if "nc" in _CACHE:
        return _CACHE["nc"]
    import concourse.bacc as bacc
    import concourse.mybir as mybir
    from concourse import tile
    from concourse.masks import make_identity

    f32 = mybir.dt.float32
    bf16 = mybir.dt.bfloat16
    mult = mybir.AluOpType.mult
    add = mybir.AluOpType.add

    nc = bacc.Bacc("TRN2", target_bir_lowering=False, debug=False, num_devices=CORES)

    x_d = nc.dram_tensor("x", [BPC, C, 2, HALF, W], bf16, kind="ExternalInput").ap()
    pw_d = nc.dram_tensor("pweight", [C, C, 3, 3, NB], bf16, kind="ExternalInput").ap()
    nw_d = nc.dram_tensor("nweight", [C, C, 3, 3, NB], bf16, kind="ExternalInput").ap()
    sc_d = nc.dram_tensor("scale", [1], f32, kind="ExternalInput").ap()
    pb_d = nc.dram_tensor("pbias", [C, NB], f32, kind="ExternalInput").ap()
    nb_d = nc.dram_tensor("nbias", [C, NB], f32, kind="ExternalInput").ap()
    bs_d = nc.dram_tensor("biasscale", [1], f32, kind="ExternalInput").ap()
    y_d = nc.dram_tensor("y", [BPC, C, 2, HALF, W], bf16, kind="ExternalOutput").ap()

    with tile.TileContext(nc) as tc:
        with (
            tc.tile_pool(name="consts", bufs=1) as consts,
            tc.tile_pool(name="xpool", bufs=XBUFS) as xpool,
            tc.tile_pool(name="opool", bufs=2) as opool,
            tc.tile_pool(name="pspool", bufs=5, space="PSUM") as pspool,
            tc.tile_pool(name="psum_c", bufs=1, space="PSUM") as psum_c,
            tc.tile_pool(name="psum_t", bufs=1, space="PSUM") as psum_t,
        ):
            ident = consts.tile([C, C], f32, tag="ident")
            make_identity(nc, ident[:])
            # HAM warmup: dummy matmuls keep the PE busy (and un-throttled)
            # while the weight planes and image 0 stream in.
            warm_w = consts.tile([128, 128], bf16, tag="warm_w")
            warm_x = consts.tile([128, NT], bf16, tag="warm_x")
            nc.gpsimd.memset(warm_w[:], 0)
            nc.gpsimd.memset(warm_x[:], 0)

            # lhsT6[s*3+u]: the 128x128 stationary operand for (s, u)
            lhsT6 = [
                consts.tile([128, 128], bf16, tag=f"lhsT6_{i}", name=f"lhsT6_{i}")
                for i in range(6)
            ]
            scale_vec = consts.tile([128, 1], f32, tag="scale_vec")
            bias_vec = consts.tile([128, 1], f32, tag="bias_vec")

            # ---- weight/bias reconstruction (tiny, runs once) ----
            wp = consts.tile([C, C * 9 * NB], bf16, tag="wp")
            wn = consts.tile([C, C * 9 * NB], bf16, tag="wn")
            nc.sync.dma_start(wp[:], pw_d.rearrange("o i kh kw b -> o (i kh kw b)"))
            nc.scalar.dma_start(wn[:], nw_d.rearrange("o i kh kw b -> o (i kh kw b)"))
            nc.vector.tensor_sub(wp[:], wp[:], wn[:])  # d = p - n (exact in bf16)
            wi = consts.tile([C, 9 * C], f32, tag="wi")
            wt2 = consts.tile([C, 9 * C], f32, tag="wt2")
            wi_v = wi[:].rearrange("p (t i) -> p t i", t=9)
            wt2_v = wt2[:].rearrange("p (t i) -> p t i", t=9)
            d_v = wp[:].rearrange("p (i t b) -> p t i b", t=9, b=NB)
            nc.vector.scalar_tensor_tensor(
                out=wt2_v, in0=d_v[:, :, :, 0], scalar=8.0, in1=d_v[:, :, :, 3],
                op0=mult, op1=add,
            )
            nc.vector.scalar_tensor_tensor(
                out=wi_v, in0=d_v[:, :, :, 1], scalar=4.0, in1=wt2_v,
                op0=mult, op1=add,
            )
            nc.vector.scalar_tensor_tensor(
                out=wt2_v, in0=d_v[:, :, :, 2], scalar=2.0, in1=wi_v,
                op0=mult, op1=add,
            )
            for i in range(6):
                nc.gpsimd.memset(lhsT6[i][:], 0)
            # t = kh*3+u; each transposed tap Wt^T lands in two 64x64 blocks:
            #   kh=0 -> s0[0:64,0:64] and s0[64:128,64:128]
            #   kh=1 -> s0[64:128,0:64] and s1[0:64,64:128]
            #   kh=2 -> s1[0:64,0:64] and s1[64:128,64:128]
            for t in range(9):
                kh, u = divmod(t, 3)
                wtmp = consts.tile([C, 128], f32, tag=f"wtmp{t % 2}", name=f"wtmp{t}")
                nc.scalar.copy(wtmp[:, 0:C], wt2_v[:, t, :])
                nc.scalar.copy(wtmp[:, C:128], wt2_v[:, t, :])
                ps = psum_t.tile([128, C], f32, tag="tps", name=f"tps{t}")
                nc.tensor.transpose(ps[:], wtmp[:], ident[:])
                if kh == 0:
                    nc.scalar.copy(lhsT6[u][0:C, 0:C], ps[0:C, :])
                    nc.scalar.copy(lhsT6[u][C:128, C:128], ps[C:128, :])
                elif kh == 1:
                    nc.scalar.copy(lhsT6[u][C:128, 0:C], ps[C:128, :])
                    nc.scalar.copy(lhsT6[3 + u][0:C, C:128], ps[0:C, :])
                else:
                    nc.scalar.copy(lhsT6[3 + u][0:C, 0:C], ps[0:C, :])
                    nc.scalar.copy(lhsT6[3 + u][C:128, C:128], ps[C:128, :])
            # bias vector, duplicated across both partition blocks
            pbt = consts.tile([128, NB], f32, tag="pbt")
            nbt = consts.tile([128, NB], f32, tag="nbt")
            nc.sync.dma_start(pbt[0:C, :], pb_d)
            nc.sync.dma_start(pbt[C:128, :], pb_d)
            nc.sync.dma_start(nbt[0:C, :], nb_d)
            nc.sync.dma_start(nbt[C:128, :], nb_d)
            nc.vector.tensor_sub(pbt[:], pbt[:], nbt[:])
            btmp = consts.tile([128, 1], f32, tag="btmp")
            nc.vector.scalar_tensor_tensor(
                out=btmp[:], in0=pbt[:, 0:1], scalar=8.0, in1=pbt[:, 3:4],
                op0=mult, op1=add,
            )
            nc.vector.scalar_tensor_tensor(
                out=bias_vec[:], in0=pbt[:, 1:2], scalar=4.0, in1=btmp[:],
                op0=mult, op1=add,
            )
            nc.vector.scalar_tensor_tensor(
                out=btmp[:], in0=pbt[:, 2:3], scalar=2.0, in1=bias_vec[:],
                op0=mult, op1=add,
            )
            bsv = consts.tile([128, 1], f32, tag="bsv")
            nc.sync.dma_start(bsv[:], bs_d.to_broadcast((128, 1)))
            nc.vector.tensor_mul(btmp[:], btmp[:], bsv[:])
            nc.scalar.mul(bias_vec[:], btmp[:], 1.0 / 15.0)
            nc.sync.dma_start(scale_vec[:], sc_d.to_broadcast((128, 1)))
            nc.scalar.mul(scale_vec[:], scale_vec[:], 1.0 / 15.0)

            # ---- one-time zeroing of the pad regions per physical buffer ----
            # block0 = [Z, x1, x3, .., x111] at parts 0:64, data at D+112
            # block1 = [x0, x2, .., x110, Z] at parts 64:128, data at D
            for i in range(XBUFS):
                xz = xpool.tile([128, XC], bf16, tag="xs", name=f"xz{i}")
                nc.gpsimd.memset(xz[0:C, 0 : D + W], 0)
                nc.gpsimd.memset(xz[0:C, D + NROW0 * W : XC], 0)
                nc.gpsimd.memset(xz[C:128, 0:D], 0)
                nc.gpsimd.memset(xz[C:128, D + OUTC : XC], 0)

            for i in range(26):
                wps = pspool.tile([128, NT], f32, tag="ps", name=f"warm{i}")
                nc.tensor.matmul(wps[:], warm_w[:], warm_x[:], start=True, stop=True)

            # ---- per-image load: contiguous HWDGE DMA of the parity planes ----
            def load_image(b):
                xs = xpool.tile([128, XC], bf16, tag="xs", name=f"xs{b}")
                for r0, r1 in IN_CHUNKS:
                    nc.sync.dma_start(
                        xs[0:C, D + (1 + r0) * W : D + (1 + r1) * W].rearrange(
                            "p (r w) -> p r w", w=W
                        ),
                        x_d[b, :, 1, r0:r1, :],
                    )
                    nc.sync.dma_start(
                        xs[C:128, D + r0 * W : D + r1 * W].rearrange(
                            "p (r w) -> p r w", w=W
                        ),
                        x_d[b, :, 0, r0:r1, :],
                    )
                return xs

            xs_list = [load_image(0)] + [None] * (BPC - 1)

            # strided [128, 56] view of columns base + m*112
            def col_view(xs, base):
                return xs[:, base : base + OUTC].rearrange(
                    "p (m w) -> p w m", w=W
                )[:, 0, :]

            # wrap-fixup: 2+2 matmuls reusing the conv stationaries; the views
            # read exactly the addresses the wrapped taps read at cols 0/111.
            def wrap_fixup(b, xs):
                corrL = psum_c.tile([128, HALF], f32, tag="corrL", name=f"corrL{b}")
                corrR = psum_c.tile([128, HALF], f32, tag="corrR", name=f"corrR{b}")
                for s in range(2):
                    nc.tensor.matmul(
                        corrL[:], lhsT6[3 * s][:], col_view(xs, s * W),
                        start=(s == 0), stop=(s == 1),
                    )
                for s in range(2):
                    nc.tensor.matmul(
                        corrR[:], lhsT6[3 * s + 2][:], col_view(xs, (s + 1) * W + D),
                        start=(s == 0), stop=(s == 1),
                    )
                tmpL = opool.tile([128, HALF], f32, tag="tmpL", name=f"tmpL{b}")
                tmpR = opool.tile([128, HALF], f32, tag="tmpR", name=f"tmpR{b}")
                nc.vector.tensor_scalar(
                    out=tmpL[:], in0=corrL[:], scalar1=scale_vec[:], scalar2=None,
                    op0=mult,
                )
                nc.vector.tensor_scalar(
                    out=tmpR[:], in0=corrR[:], scalar1=scale_vec[:], scalar2=None,
                    op0=mult,
                )
                return tmpL, tmpR

            # ---- main conv loop ----
            for b in range(BPC):
                xs = xs_list[b]
                if b >= 1 and b + 2 < BPC:
                    xs_list[b + 2] = load_image(b + 2)

                outy = opool.tile([128, OUTC], bf16, tag="outy", name=f"outy{b}")
                ove = outy[:].rearrange("p (m w) -> p w m", w=W)  # [128, 112, 56]

                if b > 0:
                    tmpL, tmpR = wrap_fixup(b, xs)

                for tp in range(0, NTILES, 2):
                    # tile pairs: each stationary load serves two matmuls
                    psA = pspool.tile([128, NT], f32, tag="ps", name=f"psA{b}_{tp}")
                    psB = pspool.tile([128, NT], f32, tag="ps", name=f"psB{b}_{tp}")
                    nA = tp * NT
                    nB = (tp + 1) * NT
                    for s in range(2):
                        for u in range(3):
                            off = s * W + u
                            first = s == 0 and u == 0
                            last = s == 1 and u == 2
                            nc.tensor.matmul(
                                psA[:], lhsT6[3 * s + u][:],
                                xs[:, nA + off : nA + off + NT],
                                start=first, stop=last,
                            )
                            nc.tensor.matmul(
                                psB[:], lhsT6[3 * s + u][:],
                                xs[:, nB + off : nB + off + NT],
                                start=first, stop=last,
                            )
                    # epilogue on DVE: scale+bias, both parities in one op
                    for n0, ps in ((nA, psA), (nB, psB)):
                        nc.vector.tensor_scalar(
                            out=outy[:, n0 : n0 + NT],
                            in0=ps[:],
                            scalar1=scale_vec[:],
                            scalar2=bias_vec[:],
                            op0=mult,
                            op1=add,
                        )
                    if b == 0 and tp == 2:
                        xs_list[1] = load_image(1)
                    if b == 0 and tp == 8:
                        xs_list[2] = load_image(2)
                    # stream out behind the epilogue: fix wrap cols, store.
                    # Steady images store in 2 groups; the last image per
                    # pair (8 row-pairs) to shrink the drain.
                    if b == BPC - 1:
                        bounds = [(tp, tp * 4, 8) for tp in range(0, NTILES, 2)]
                    else:
                        bounds = [(6, 0, 32), (12, 32, 24)]
                    if b > 0:
                        for tpb, r0, nr in bounds:
                            if tp != tpb:
                                continue
                            nc.vector.tensor_sub(
                                ove[:, 0, r0 : r0 + nr],
                                ove[:, 0, r0 : r0 + nr],
                                tmpL[:, r0 : r0 + nr],
                            )
                            nc.vector.tensor_sub(
                                ove[:, 111, r0 : r0 + nr],
                                ove[:, 111, r0 : r0 + nr],
                                tmpR[:, r0 : r0 + nr],
                            )
                            for pl, p0 in ((0, 0), (1, C)):
                                nc.scalar.dma_start(
                                    y_d[b, :, pl, r0 : r0 + nr, :],
                                    outy[
                                        p0 : p0 + C, r0 * W : (r0 + nr) * W
                                    ].rearrange("p (r w) -> p r w", w=W),
                                )
                if b == 0:
                    tmpL, tmpR = wrap_fixup(b, xs)
                    nc.vector.tensor_sub(ove[:, 0, :], ove[:, 0, :], tmpL[:])
                    nc.vector.tensor_sub(ove[:, 111, :], ove[:, 111, :], tmpR[:])
                    for r0 in range(0, HALF, 28):
                        for pl, p0 in ((0, 0), (1, C)):
                            nc.scalar.dma_start(
                                y_d[b, :, pl, r0 : r0 + 28, :],
                                outy[p0 : p0 + C, r0 * W : (r0 + 28) * W].rearrange(
                                    "p (r w) -> p r w", w=W
                                ),
                            )

    nc.compile()
    _CACHE["nc"] = nc
    return nc


def _run(inputs, trace=False):
    import ml_dtypes
    from concourse.bass_utils import run_bass_kernel_spmd

    nc = _build()
    # host-side: bf16 + row-parity-plane layout [B, C, 2, 56, W]
    x = (
        np.asarray(inputs["x"], dtype=np.float32)
        .astype(ml_dtypes.bfloat16)
        .reshape(B, C, HALF, 2, W)
        .transpose(0, 1, 3, 2, 4)
    )
    x = np.ascontiguousarray(x)
    shared = {
        "pweight": np.ascontiguousarray(
            np.asarray(inputs["pweight"], np.float32).astype(ml_dtypes.bfloat16)
        ),
        "nweight": np.ascontiguousarray(
            np.asarray(inputs["nweight"], np.float32).astype(ml_dtypes.bfloat16)
        ),
        "scale": np.ascontiguousarray(np.asarray(inputs["scale"], np.float32)),
        "pbias": np.ascontiguousarray(np.asarray(inputs["pbias"], np.float32)),
        "nbias": np.ascontiguousarray(np.asarray(inputs["nbias"], np.float32)),
        "biasscale": np.ascontiguousarray(np.asarray(inputs["biasscale"], np.float32)),
    }
    in_maps = [dict(shared, x=x[c * BPC : (c + 1) * BPC]) for c in range(CORES)]
    last_err = None
    for attempt in range(3):
        try:
            res = run_bass_kernel_spmd(
                nc, in_maps, core_ids=list(range(CORES)), trace=trace
            )
            y = np.concatenate(
                [np.asarray(res.results[c]["y"]) for c in range(CORES)], axis=0
            )
            # undo the parity-plane layout, upcast
            out = (
                y.reshape(B, C, 2, HALF, W)
                .transpose(0, 1, 3, 2, 4)
                .reshape(B, C, H, W)
                .astype(np.float32)
            )
            return np.ascontiguousarray(out), res.exec_time_ns
        except Exception as e:  # transient NRT_EXEC_UNIT_UNRECOVERABLE recovers on retry
            last_err = e
            import time

            time.sleep(10)
    raise last_err


def kernel(**inputs) -> np.ndarray:
    out, _ = _run(inputs)
    return out


# revision 26
# speedup vs baseline: 1.1761x; 1.1761x over previous
"""BitConv2d forward on 8 Trainium2 NeuronCores (SPMD data-parallel).

Strategy (v9 -- even/odd row-parity K-packing):
  - Shard batch (32) -> 4 images per core; replicate the tiny bit-plane
    weights/scales on every core. No collectives needed (forward only).
  - x and y move through HBM as bf16 AND in row-parity-plane layout
    [B, C, 2, 56, W] (host numpy pre/post shuffles -- pure data layout prep,
    no conv math on the host). Precision ~4e-3 max rel err vs the 2e-2 gate.
  - The parity layout packs the PE contraction dim: partitions 0:64 hold the
    EVEN padded rows of the image (cin-major), partitions 64:128 the ODD
    padded rows. One 128x128 stationary operand then carries TWO vertical
    taps for BOTH output-row parities (3 of its 4 64x64 blocks non-zero), so
    the 3x3 conv needs 6 accumulating matmuls per output tile instead of 9:
       s=0,u: [[Wt(0,u), 0], [Wt(1,u), Wt(0,u)]]
       s=1,u: [[Wt(2,u), Wt(1,u)], [0, Wt(2,u)]]   (K-blocks x M-parities)
    75% PE utilization vs 50% for the classic block-diagonal halves scheme.
  - NO column padding: rows are stored 112-contiguous, horizontal taps wrap
    across row boundaries, and the wrap contributions are cancelled exactly
    by 4 small fixup matmuls per image (N=56 stride-112 views, reusing the
    same stationary tiles) subtracted at output cols 0 and 111.
  - Every HBM<->SBUF transfer is large contiguous descriptors; all DMA on
    HWDGE (input on the sync ring, output on the scalar ring). PSUM tiles
    N=448 = 4 row-pairs = 8 output rows; epilogue (scale+bias, f32 psum ->
    bf16) is a single contiguous DVE op per tile; output streams out behind
    the epilogue in row-pair groups.
  - Dummy matmuls at kernel start keep the PE HAM-warm through the weight
    load so the first real tile runs at 2.4 GHz.
"""

import numpy as np

B, C, H, W = 32, 64, 112, 112
NB = 4
CORES = 8
BPC = B // CORES  # images per core

HALF = H // 2  # 56 row-pairs (and 56 rows per output plane)
D = 1  # data base column (one zero col in front)
NROW0 = 57  # block rows incl the zero pad row
XC = D + NROW0 * W + 115  # 6500 total cols
OUTC = HALF * W  # 6272 output cols per partition (one parity plane)

NT = 448  # = 4*112: one PSUM tile covers 4 row-pairs = 8 output rows
NTILES = 14  # 14*448 = 6272
XBUFS = 3

# input chunks in row-pair units (conv tile t needs block rows <= 4t+4)
IN_CHUNKS = [(0, 19), (19, 38), (38, 56)]

_CACHE = {}


def _build():
    if "nc" in _CACHE:
        return _CACHE["nc"]
    import concourse.bacc as bacc
    import concourse.mybir as mybir
    from concourse import tile
    from concourse.masks import make_identity

    f32 = mybir.dt.float32
    bf16 = mybir.dt.bfloat16
    fp8 = mybir.dt.float8e3
    mult = mybir.AluOpType.mult
    add = mybir.AluOpType.add

    nc = bacc.Bacc("TRN2", target_bir_lowering=False, debug=False, num_devices=CORES)

    x_d = nc.dram_tensor("x", [BPC, C, 2, HALF, W], fp8, kind="ExternalInput").ap()
    pw_d = nc.dram_tensor("pweight", [C, C, 3, 3, NB], bf16, kind="ExternalInput").ap()
    nw_d = nc.dram_tensor("nweight", [C, C, 3, 3, NB], bf16, kind="ExternalInput").ap()
    sc_d = nc.dram_tensor("scale", [1], f32, kind="ExternalInput").ap()
    pb_d = nc.dram_tensor("pbias", [C, NB], f32, kind="ExternalInput").ap()
    nb_d = nc.dram_tensor("nbias", [C, NB], f32, kind="ExternalInput").ap()
    bs_d = nc.dram_tensor("biasscale", [1], f32, kind="ExternalInput").ap()
    y_d = nc.dram_tensor("y", [BPC, C, 2, HALF, W], bf16, kind="ExternalOutput").ap()

    with tile.TileContext(nc) as tc:
        with (
            tc.tile_pool(name="consts", bufs=1) as consts,
            tc.tile_pool(name="xpool", bufs=XBUFS) as xpool,
            tc.tile_pool(name="opool", bufs=2) as opool,
            tc.tile_pool(name="pspool", bufs=5, space="PSUM") as pspool,
            tc.tile_pool(name="psum_c", bufs=1, space="PSUM") as psum_c,
            tc.tile_pool(name="psum_t", bufs=1, space="PSUM") as psum_t,
        ):
            ident = consts.tile([C, C], f32, tag="ident")
            make_identity(nc, ident[:])
            # HAM warmup: dummy matmuls keep the PE busy (and un-throttled)
            # while the weight planes and image 0 stream in.
            warm_w = consts.tile([128, 128], fp8, tag="warm_w")
            warm_x = consts.tile([128, NT], fp8, tag="warm_x")
            nc.gpsimd.memset(warm_w[:], 0)
            nc.gpsimd.memset(warm_x[:], 0)

            # lhsT6[s*3+u]: the 128x128 stationary operand for (s, u)
            lhsT6 = [
                consts.tile([128, 128], fp8, tag=f"lhsT6_{i}", name=f"lhsT6_{i}")
                for i in range(6)
            ]
            scale_vec = consts.tile([128, 1], f32, tag="scale_vec")
            bias_vec = consts.tile([128, 1], f32, tag="bias_vec")

            # ---- weight/bias reconstruction (tiny, runs once) ----
            wp = consts.tile([C, C * 9 * NB], bf16, tag="wp")
            wn = consts.tile([C, C * 9 * NB], bf16, tag="wn")
            nc.sync.dma_start(wp[:], pw_d.rearrange("o i kh kw b -> o (i kh kw b)"))
            nc.scalar.dma_start(wn[:], nw_d.rearrange("o i kh kw b -> o (i kh kw b)"))
            nc.vector.tensor_sub(wp[:], wp[:], wn[:])  # d = p - n (exact in bf16)
            wi = consts.tile([C, 9 * C], f32, tag="wi")
            wt2 = consts.tile([C, 9 * C], f32, tag="wt2")
            wi_v = wi[:].rearrange("p (t i) -> p t i", t=9)
            wt2_v = wt2[:].rearrange("p (t i) -> p t i", t=9)
            d_v = wp[:].rearrange("p (i t b) -> p t i b", t=9, b=NB)
            nc.vector.scalar_tensor_tensor(
                out=wt2_v, in0=d_v[:, :, :, 0], scalar=8.0, in1=d_v[:, :, :, 3],
                op0=mult, op1=add,
            )
            nc.vector.scalar_tensor_tensor(
                out=wi_v, in0=d_v[:, :, :, 1], scalar=4.0, in1=wt2_v,
                op0=mult, op1=add,
            )
            nc.vector.scalar_tensor_tensor(
                out=wt2_v, in0=d_v[:, :, :, 2], scalar=2.0, in1=wi_v,
                op0=mult, op1=add,
            )
            for i in range(6):
                nc.gpsimd.memset(lhsT6[i][:], 0)
            # t = kh*3+u; each transposed tap Wt^T lands in two 64x64 blocks:
            #   kh=0 -> s0[0:64,0:64] and s0[64:128,64:128]
            #   kh=1 -> s0[64:128,0:64] and s1[0:64,64:128]
            #   kh=2 -> s1[0:64,0:64] and s1[64:128,64:128]
            for t in range(9):
                kh, u = divmod(t, 3)
                wtmp = consts.tile([C, 128], f32, tag=f"wtmp{t % 2}", name=f"wtmp{t}")
                nc.scalar.copy(wtmp[:, 0:C], wt2_v[:, t, :])
                nc.scalar.copy(wtmp[:, C:128], wt2_v[:, t, :])
                ps = psum_t.tile([128, C], f32, tag="tps", name=f"tps{t}")
                nc.tensor.transpose(ps[:], wtmp[:], ident[:])
                if kh == 0:
                    nc.scalar.copy(lhsT6[u][0:C, 0:C], ps[0:C, :])
                    nc.scalar.copy(lhsT6[u][C:128, C:128], ps[C:128, :])
                elif kh == 1:
                    nc.scalar.copy(lhsT6[u][C:128, 0:C], ps[C:128, :])
                    nc.scalar.copy(lhsT6[3 + u][0:C, C:128], ps[0:C, :])
                else:
                    nc.scalar.copy(lhsT6[3 + u][0:C, 0:C], ps[0:C, :])
                    nc.scalar.copy(lhsT6[3 + u][C:128, C:128], ps[C:128, :])
            # bias vector, duplicated across both partition blocks
            pbt = consts.tile([128, NB], f32, tag="pbt")
            nbt = consts.tile([128, NB], f32, tag="nbt")
            nc.sync.dma_start(pbt[0:C, :], pb_d)
            nc.sync.dma_start(pbt[C:128, :], pb_d)
            nc.sync.dma_start(nbt[0:C, :], nb_d)
            nc.sync.dma_start(nbt[C:128, :], nb_d)
            nc.vector.tensor_sub(pbt[:], pbt[:], nbt[:])
            btmp = consts.tile([128, 1], f32, tag="btmp")
            nc.vector.scalar_tensor_tensor(
                out=btmp[:], in0=pbt[:, 0:1], scalar=8.0, in1=pbt[:, 3:4],
                op0=mult, op1=add,
            )
            nc.vector.scalar_tensor_tensor(
                out=bias_vec[:], in0=pbt[:, 1:2], scalar=4.0, in1=btmp[:],
                op0=mult, op1=add,
            )
            nc.vector.scalar_tensor_tensor(
                out=btmp[:], in0=pbt[:, 2:3], scalar=2.0, in1=bias_vec[:],
                op0=mult, op1=add,
            )
            bsv = consts.tile([128, 1], f32, tag="bsv")
            nc.sync.dma_start(bsv[:], bs_d.to_broadcast((128, 1)))
            nc.vector.tensor_mul(btmp[:], btmp[:], bsv[:])
            nc.scalar.mul(bias_vec[:], btmp[:], 1.0 / 15.0)
            nc.sync.dma_start(scale_vec[:], sc_d.to_broadcast((128, 1)))
            nc.scalar.mul(scale_vec[:], scale_vec[:], 1.0 / 15.0)

            # ---- one-time zeroing of the pad regions per physical buffer ----
            # block0 = [Z, x1, x3, .., x111] at parts 0:64, data at D+112
            # block1 = [x0, x2, .., x110, Z] at parts 64:128, data at D
            for i in range(XBUFS):
                xz = xpool.tile([128, XC], fp8, tag="xs", name=f"xz{i}")
                nc.gpsimd.memset(xz[0:C, 0 : D + W], 0)
                nc.gpsimd.memset(xz[0:C, D + NROW0 * W : XC], 0)
                nc.gpsimd.memset(xz[C:128, 0:D], 0)
                nc.gpsimd.memset(xz[C:128, D + OUTC : XC], 0)

            for i in range(26):
                wps = pspool.tile([128, NT], f32, tag="ps", name=f"warm{i}")
                nc.tensor.matmul(wps[:], warm_w[:], warm_x[:], start=True, stop=True)

            # ---- per-image load: contiguous HWDGE DMA of the parity planes ----
            def load_image(b):
                xs = xpool.tile([128, XC], fp8, tag="xs", name=f"xs{b}")
                for r0, r1 in IN_CHUNKS:
                    nc.sync.dma_start(
                        xs[0:C, D + (1 + r0) * W : D + (1 + r1) * W].rearrange(
                            "p (r w) -> p r w", w=W
                        ),
                        x_d[b, :, 1, r0:r1, :],
                    )
                    nc.sync.dma_start(
                        xs[C:128, D + r0 * W : D + r1 * W].rearrange(
                            "p (r w) -> p r w", w=W
                        ),
                        x_d[b, :, 0, r0:r1, :],
                    )
                return xs

            xs_list = [load_image(0)] + [None] * (BPC - 1)

            # strided [128, 56] view of columns base + m*112
            def col_view(xs, base):
                return xs[:, base : base + OUTC].rearrange(
                    "p (m w) -> p w m", w=W
                )[:, 0, :]

            # wrap-fixup: 2+2 matmuls reusing the conv stationaries; the views
            # read exactly the addresses the wrapped taps read at cols 0/111.
            def wrap_fixup(b, xs):
                corrL = psum_c.tile([128, HALF], f32, tag="corrL", name=f"corrL{b}")
                corrR = psum_c.tile([128, HALF], f32, tag="corrR", name=f"corrR{b}")
                for s in range(2):
                    nc.tensor.matmul(
                        corrL[:], lhsT6[3 * s][:], col_view(xs, s * W),
                        start=(s == 0), stop=(s == 1),
                    )
                for s in range(2):
                    nc.tensor.matmul(
                        corrR[:], lhsT6[3 * s + 2][:], col_view(xs, (s + 1) * W + D),
                        start=(s == 0), stop=(s == 1),
                    )
                tmpL = opool.tile([128, HALF], f32, tag="tmpL", name=f"tmpL{b}")
                tmpR = opool.tile([128, HALF], f32, tag="tmpR", name=f"tmpR{b}")
                nc.vector.tensor_scalar(
                    out=tmpL[:], in0=corrL[:], scalar1=scale_vec[:], scalar2=None,
                    op0=mult,
                )
                nc.vector.tensor_scalar(
                    out=tmpR[:], in0=corrR[:], scalar1=scale_vec[:], scalar2=None,
                    op0=mult,
                )
                return tmpL, tmpR

            # ---- main conv loop ----
            for b in range(BPC):
                xs = xs_list[b]
                if b >= 1 and b + 2 < BPC:
                    xs_list[b + 2] = load_image(b + 2)

                outy = opool.tile([128, OUTC], bf16, tag="outy", name=f"outy{b}")
                ove = outy[:].rearrange("p (m w) -> p w m", w=W)  # [128, 112, 56]

                if b > 0:
                    tmpL, tmpR = wrap_fixup(b, xs)

                for tp in range(0, NTILES, 2):
                    # tile pairs: each stationary load serves two matmuls
                    psA = pspool.tile([128, NT], f32, tag="ps", name=f"psA{b}_{tp}")
                    psB = pspool.tile([128, NT], f32, tag="ps", name=f"psB{b}_{tp}")
                    nA = tp * NT
                    nB = (tp + 1) * NT
                    for s in range(2):
                        for u in range(3):
                            off = s * W + u
                            first = s == 0 and u == 0
                            last = s == 1 and u == 2
                            nc.tensor.matmul(
                                psA[:], lhsT6[3 * s + u][:],
                                xs[:, nA + off : nA + off + NT],
                                start=first, stop=last,
                            )
                            nc.tensor.matmul(
                                psB[:], lhsT6[3 * s + u][:],
                                xs[:, nB + off : nB + off + NT],
                                start=first, stop=last,
                            )
                    # epilogue on DVE: scale+bias, both parities in one op
                    for n0, ps in ((nA, psA), (nB, psB)):
                        nc.vector.tensor_scalar(
                            out=outy[:, n0 : n0 + NT],
                            in0=ps[:],
                            scalar1=scale_vec[:],
                            scalar2=bias_vec[:],
                            op0=mult,
                            op1=add,
                        )
                    if b == 0 and tp == 2:
                        xs_list[1] = load_image(1)
                    if b == 0 and tp == 8:
                        xs_list[2] = load_image(2)
                    # stream out behind the epilogue: fix wrap cols, store.
                    # Steady images store in 2 groups; the last image per
                    # pair (8 row-pairs) to shrink the drain.
                    if b == BPC - 1:
                        bounds = [(tp, tp * 4, 8) for tp in range(0, NTILES, 2)]
                    else:
                        bounds = [(6, 0, 32), (12, 32, 24)]
                    if b > 0:
                        for tpb, r0, nr in bounds:
                            if tp != tpb:
                                continue
                            nc.vector.tensor_sub(
                                ove[:, 0, r0 : r0 + nr],
                                ove[:, 0, r0 : r0 + nr],
                                tmpL[:, r0 : r0 + nr],
                            )
                            nc.vector.tensor_sub(
                                ove[:, 111, r0 : r0 + nr],
                                ove[:, 111, r0 : r0 + nr],
                                tmpR[:, r0 : r0 + nr],
                            )
                            for pl, p0 in ((0, 0), (1, C)):
                                nc.scalar.dma_start(
                                    y_d[b, :, pl, r0 : r0 + nr, :],
                                    outy[
                                        p0 : p0 + C, r0 * W : (r0 + nr) * W
                                    ].rearrange("p (r w) -> p r w", w=W),
                                )
                if b == 0:
                    tmpL, tmpR = wrap_fixup(b, xs)
                    nc.vector.tensor_sub(ove[:, 0, :], ove[:, 0, :], tmpL[:])
                    nc.vector.tensor_sub(ove[:, 111, :], ove[:, 111, :], tmpR[:])
                    for r0 in range(0, HALF, 28):
                        for pl, p0 in ((0, 0), (1, C)):
                            nc.scalar.dma_start(
                                y_d[b, :, pl, r0 : r0 + 28, :],
                                outy[p0 : p0 + C, r0 * W : (r0 + 28) * W].rearrange(
                                    "p (r w) -> p r w", w=W
                                ),
                            )

    nc.compile()
    _CACHE["nc"] = nc
    return nc


def _run(inputs, trace=False):
    import ml_dtypes
    from concourse.bass_utils import run_bass_kernel_spmd

    nc = _build()
    # host-side: bf16 + row-parity-plane layout [B, C, 2, 56, W]
    x = (
        np.asarray(inputs["x"], dtype=np.float32)
        .astype(ml_dtypes.float8_e3m4)
        .reshape(B, C, HALF, 2, W)
        .transpose(0, 1, 3, 2, 4)
    )
    x = np.ascontiguousarray(x)
    shared = {
        "pweight": np.ascontiguousarray(
            np.asarray(inputs["pweight"], np.float32).astype(ml_dtypes.bfloat16)
        ),
        "nweight": np.ascontiguousarray(
            np.asarray(inputs["nweight"], np.float32).astype(ml_dtypes.bfloat16)
        ),
        "scale": np.ascontiguousarray(np.asarray(inputs["scale"], np.float32)),
        "pbias": np.ascontiguousarray(np.asarray(inputs["pbias"], np.float32)),
        "nbias": np.ascontiguousarray(np.asarray(inputs["nbias"], np.float32)),
        "biasscale": np.ascontiguousarray(np.asarray(inputs["biasscale"], np.float32)),
    }
    in_maps = [dict(shared, x=x[c * BPC : (c + 1) * BPC]) for c in range(CORES)]
    last_err = None
    for attempt in range(3):
        try:
            res = run_bass_kernel_spmd(
                nc, in_maps, core_ids=list(range(CORES)), trace=trace
            )
            y = np.concatenate(
                [np.asarray(res.results[c]["y"]) for c in range(CORES)], axis=0
            )
            # undo the parity-plane layout, upcast
            out = (
                y.reshape(B, C, 2, HALF, W)
                .transpose(0, 1, 3, 2, 4)
                .reshape(B, C, H, W)
                .astype(np.float32)
            )
            return np.ascontiguousarray(out), res.exec_time_ns
        except Exception as e:  # transient NRT_EXEC_UNIT_UNRECOVERABLE recovers on retry
            last_err = e
            import time

            time.sleep(10)
    raise last_err


def kernel(**inputs) -> np.ndarray:
    out, _ = _run(inputs)
    return out


# revision 27
# speedup vs baseline: 1.1829x; 1.0058x over previous
"""BitConv2d forward on 8 Trainium2 NeuronCores (SPMD data-parallel).

Strategy (v9 -- even/odd row-parity K-packing):
  - Shard batch (32) -> 4 images per core; replicate the tiny bit-plane
    weights/scales on every core. No collectives needed (forward only).
  - x and y move through HBM as bf16 AND in row-parity-plane layout
    [B, C, 2, 56, W] (host numpy pre/post shuffles -- pure data layout prep,
    no conv math on the host). Precision ~4e-3 max rel err vs the 2e-2 gate.
  - The parity layout packs the PE contraction dim: partitions 0:64 hold the
    EVEN padded rows of the image (cin-major), partitions 64:128 the ODD
    padded rows. One 128x128 stationary operand then carries TWO vertical
    taps for BOTH output-row parities (3 of its 4 64x64 blocks non-zero), so
    the 3x3 conv needs 6 accumulating matmuls per output tile instead of 9:
       s=0,u: [[Wt(0,u), 0], [Wt(1,u), Wt(0,u)]]
       s=1,u: [[Wt(2,u), Wt(1,u)], [0, Wt(2,u)]]   (K-blocks x M-parities)
    75% PE utilization vs 50% for the classic block-diagonal halves scheme.
  - NO column padding: rows are stored 112-contiguous, horizontal taps wrap
    across row boundaries, and the wrap contributions are cancelled exactly
    by 4 small fixup matmuls per image (N=56 stride-112 views, reusing the
    same stationary tiles) subtracted at output cols 0 and 111.
  - Every HBM<->SBUF transfer is large contiguous descriptors; all DMA on
    HWDGE (input on the sync ring, output on the scalar ring). PSUM tiles
    N=448 = 4 row-pairs = 8 output rows; epilogue (scale+bias, f32 psum ->
    bf16) is a single contiguous DVE op per tile; output streams out behind
    the epilogue in row-pair groups.
  - Dummy matmuls at kernel start keep the PE HAM-warm through the weight
    load so the first real tile runs at 2.4 GHz.
"""

import numpy as np

B, C, H, W = 32, 64, 112, 112
NB = 4
CORES = 8
BPC = B // CORES  # images per core

HALF = H // 2  # 56 row-pairs (and 56 rows per output plane)
D = 1  # data base column (one zero col in front)
NROW0 = 57  # block rows incl the zero pad row
XC = D + NROW0 * W + 115  # 6500 total cols
OUTC = HALF * W  # 6272 output cols per partition (one parity plane)

NT = 448  # = 4*112: one PSUM tile covers 4 row-pairs = 8 output rows
NTILES = 14  # 14*448 = 6272
XBUFS = 3

# input chunks in row-pair units (conv tile t needs block rows <= 4t+4)
IN_CHUNKS = [(0, 19), (19, 38), (38, 56)]

_CACHE = {}


def _build():
    if "nc" in _CACHE:
        return _CACHE["nc"]
    import concourse.bacc as bacc
    import concourse.mybir as mybir
    from concourse import tile
    from concourse.masks import make_identity

    f32 = mybir.dt.float32
    bf16 = mybir.dt.bfloat16
    mult = mybir.AluOpType.mult
    add = mybir.AluOpType.add

    nc = bacc.Bacc("TRN2", target_bir_lowering=False, debug=False, num_devices=CORES)

    x_d = nc.dram_tensor("x", [BPC, C, 2, HALF, W], bf16, kind="ExternalInput").ap()
    pw_d = nc.dram_tensor("pweight", [C, C, 3, 3, NB], bf16, kind="ExternalInput").ap()
    nw_d = nc.dram_tensor("nweight", [C, C, 3, 3, NB], bf16, kind="ExternalInput").ap()
    sc_d = nc.dram_tensor("scale", [1], f32, kind="ExternalInput").ap()
    pb_d = nc.dram_tensor("pbias", [C, NB], f32, kind="ExternalInput").ap()
    nb_d = nc.dram_tensor("nbias", [C, NB], f32, kind="ExternalInput").ap()
    bs_d = nc.dram_tensor("biasscale", [1], f32, kind="ExternalInput").ap()
    y_d = nc.dram_tensor("y", [BPC, C, 2, HALF, W], bf16, kind="ExternalOutput").ap()

    with tile.TileContext(nc) as tc:
        with (
            tc.tile_pool(name="consts", bufs=1) as consts,
            tc.tile_pool(name="xpool", bufs=XBUFS) as xpool,
            tc.tile_pool(name="opool", bufs=2) as opool,
            tc.tile_pool(name="pspool", bufs=5, space="PSUM") as pspool,
            tc.tile_pool(name="psum_c", bufs=1, space="PSUM") as psum_c,
            tc.tile_pool(name="psum_t", bufs=1, space="PSUM") as psum_t,
        ):
            ident = consts.tile([C, C], f32, tag="ident")
            make_identity(nc, ident[:])
            # HAM warmup: dummy matmuls keep the PE busy (and un-throttled)
            # while the weight planes and image 0 stream in.
            warm_w = consts.tile([128, 128], bf16, tag="warm_w")
            warm_x = consts.tile([128, NT], bf16, tag="warm_x")
            nc.gpsimd.memset(warm_w[:], 0)
            nc.gpsimd.memset(warm_x[:], 0)

            # lhsT6[s*3+u]: the 128x128 stationary operand for (s, u)
            lhsT6 = [
                consts.tile([128, 128], bf16, tag=f"lhsT6_{i}", name=f"lhsT6_{i}")
                for i in range(6)
            ]
            scale_vec = consts.tile([128, 1], f32, tag="scale_vec")
            bias_vec = consts.tile([128, 1], f32, tag="bias_vec")

            # ---- weight/bias reconstruction (tiny, runs once) ----
            wp = consts.tile([C, C * 9 * NB], bf16, tag="wp")
            wn = consts.tile([C, C * 9 * NB], bf16, tag="wn")
            nc.sync.dma_start(wp[:], pw_d.rearrange("o i kh kw b -> o (i kh kw b)"))
            nc.scalar.dma_start(wn[:], nw_d.rearrange("o i kh kw b -> o (i kh kw b)"))
            nc.vector.tensor_sub(wp[:], wp[:], wn[:])  # d = p - n (exact in bf16)
            wi = consts.tile([C, 9 * C], f32, tag="wi")
            wt2 = consts.tile([C, 9 * C], f32, tag="wt2")
            wi_v = wi[:].rearrange("p (t i) -> p t i", t=9)
            wt2_v = wt2[:].rearrange("p (t i) -> p t i", t=9)
            d_v = wp[:].rearrange("p (i t b) -> p t i b", t=9, b=NB)
            nc.vector.scalar_tensor_tensor(
                out=wt2_v, in0=d_v[:, :, :, 0], scalar=8.0, in1=d_v[:, :, :, 3],
                op0=mult, op1=add,
            )
            nc.vector.scalar_tensor_tensor(
                out=wi_v, in0=d_v[:, :, :, 1], scalar=4.0, in1=wt2_v,
                op0=mult, op1=add,
            )
            nc.vector.scalar_tensor_tensor(
                out=wt2_v, in0=d_v[:, :, :, 2], scalar=2.0, in1=wi_v,
                op0=mult, op1=add,
            )
            for i in range(6):
                nc.gpsimd.memset(lhsT6[i][:], 0)
            # t = kh*3+u; each transposed tap Wt^T lands in two 64x64 blocks:
            #   kh=0 -> s0[0:64,0:64] and s0[64:128,64:128]
            #   kh=1 -> s0[64:128,0:64] and s1[0:64,64:128]
            #   kh=2 -> s1[0:64,0:64] and s1[64:128,64:128]
            for t in range(9):
                kh, u = divmod(t, 3)
                wtmp = consts.tile([C, 128], f32, tag=f"wtmp{t % 2}", name=f"wtmp{t}")
                nc.scalar.copy(wtmp[:, 0:C], wt2_v[:, t, :])
                nc.scalar.copy(wtmp[:, C:128], wt2_v[:, t, :])
                ps = psum_t.tile([128, C], f32, tag="tps", name=f"tps{t}")
                nc.tensor.transpose(ps[:], wtmp[:], ident[:])
                if kh == 0:
                    nc.scalar.copy(lhsT6[u][0:C, 0:C], ps[0:C, :])
                    nc.scalar.copy(lhsT6[u][C:128, C:128], ps[C:128, :])
                elif kh == 1:
                    nc.scalar.copy(lhsT6[u][C:128, 0:C], ps[C:128, :])
                    nc.scalar.copy(lhsT6[3 + u][0:C, C:128], ps[0:C, :])
                else:
                    nc.scalar.copy(lhsT6[3 + u][0:C, 0:C], ps[0:C, :])
                    nc.scalar.copy(lhsT6[3 + u][C:128, C:128], ps[C:128, :])
            # bias vector, duplicated across both partition blocks
            pbt = consts.tile([128, NB], f32, tag="pbt")
            nbt = consts.tile([128, NB], f32, tag="nbt")
            nc.sync.dma_start(pbt[0:C, :], pb_d)
            nc.sync.dma_start(pbt[C:128, :], pb_d)
            nc.sync.dma_start(nbt[0:C, :], nb_d)
            nc.sync.dma_start(nbt[C:128, :], nb_d)
            nc.vector.tensor_sub(pbt[:], pbt[:], nbt[:])
            btmp = consts.tile([128, 1], f32, tag="btmp")
            nc.vector.scalar_tensor_tensor(
                out=btmp[:], in0=pbt[:, 0:1], scalar=8.0, in1=pbt[:, 3:4],
                op0=mult, op1=add,
            )
            nc.vector.scalar_tensor_tensor(
                out=bias_vec[:], in0=pbt[:, 1:2], scalar=4.0, in1=btmp[:],
                op0=mult, op1=add,
            )
            nc.vector.scalar_tensor_tensor(
                out=btmp[:], in0=pbt[:, 2:3], scalar=2.0, in1=bias_vec[:],
                op0=mult, op1=add,
            )
            bsv = consts.tile([128, 1], f32, tag="bsv")
            nc.sync.dma_start(bsv[:], bs_d.to_broadcast((128, 1)))
            nc.vector.tensor_mul(btmp[:], btmp[:], bsv[:])
            nc.scalar.mul(bias_vec[:], btmp[:], 1.0 / 15.0)
            nc.sync.dma_start(scale_vec[:], sc_d.to_broadcast((128, 1)))
            nc.scalar.mul(scale_vec[:], scale_vec[:], 1.0 / 15.0)

            # ---- one-time zeroing of the pad regions per physical buffer ----
            # block0 = [Z, x1, x3, .., x111] at parts 0:64, data at D+112
            # block1 = [x0, x2, .., x110, Z] at parts 64:128, data at D
            for i in range(XBUFS):
                xz = xpool.tile([128, XC], bf16, tag="xs", name=f"xz{i}")
                nc.gpsimd.memset(xz[0:C, 0 : D + W], 0)
                nc.gpsimd.memset(xz[0:C, D + NROW0 * W : XC], 0)
                nc.gpsimd.memset(xz[C:128, 0:D], 0)
                nc.gpsimd.memset(xz[C:128, D + OUTC : XC], 0)

            for i in range(26):
                wps = pspool.tile([128, NT], f32, tag="ps", name=f"warm{i}")
                nc.tensor.matmul(wps[:], warm_w[:], warm_x[:], start=True, stop=True)

            # ---- per-image load: contiguous HWDGE DMA of the parity planes ----
            def load_image(b):
                xs = xpool.tile([128, XC], bf16, tag="xs", name=f"xs{b}")
                for r0, r1 in IN_CHUNKS:
                    nc.sync.dma_start(
                        xs[0:C, D + (1 + r0) * W : D + (1 + r1) * W].rearrange(
                            "p (r w) -> p r w", w=W
                        ),
                        x_d[b, :, 1, r0:r1, :],
                    )
                    nc.sync.dma_start(
                        xs[C:128, D + r0 * W : D + r1 * W].rearrange(
                            "p (r w) -> p r w", w=W
                        ),
                        x_d[b, :, 0, r0:r1, :],
                    )
                return xs

            xs_list = [load_image(0)] + [None] * (BPC - 1)

            # strided [128, 56] view of columns base + m*112
            def col_view(xs, base):
                return xs[:, base : base + OUTC].rearrange(
                    "p (m w) -> p w m", w=W
                )[:, 0, :]

            # wrap-fixup: 2+2 matmuls reusing the conv stationaries; the views
            # read exactly the addresses the wrapped taps read at cols 0/111.
            def wrap_fixup(b, xs):
                corrL = psum_c.tile([128, HALF], f32, tag="corrL", name=f"corrL{b}")
                corrR = psum_c.tile([128, HALF], f32, tag="corrR", name=f"corrR{b}")
                for s in range(2):
                    nc.tensor.matmul(
                        corrL[:], lhsT6[3 * s][:], col_view(xs, s * W),
                        start=(s == 0), stop=(s == 1),
                    )
                for s in range(2):
                    nc.tensor.matmul(
                        corrR[:], lhsT6[3 * s + 2][:], col_view(xs, (s + 1) * W + D),
                        start=(s == 0), stop=(s == 1),
                    )
                tmpL = opool.tile([128, HALF], f32, tag="tmpL", name=f"tmpL{b}")
                tmpR = opool.tile([128, HALF], f32, tag="tmpR", name=f"tmpR{b}")
                nc.vector.tensor_scalar(
                    out=tmpL[:], in0=corrL[:], scalar1=scale_vec[:], scalar2=None,
                    op0=mult,
                )
                nc.vector.tensor_scalar(
                    out=tmpR[:], in0=corrR[:], scalar1=scale_vec[:], scalar2=None,
                    op0=mult,
                )
                return tmpL, tmpR

            # ---- main conv loop ----
            for b in range(BPC):
                xs = xs_list[b]
                if b >= 1 and b + 2 < BPC:
                    xs_list[b + 2] = load_image(b + 2)

                outy = opool.tile([128, OUTC], bf16, tag="outy", name=f"outy{b}")
                ove = outy[:].rearrange("p (m w) -> p w m", w=W)  # [128, 112, 56]

                if b > 0:
                    tmpL, tmpR = wrap_fixup(b, xs)

                for tp in range(0, NTILES, 2):
                    # tile pairs: each stationary load serves two matmuls
                    psA = pspool.tile([128, NT], f32, tag="ps", name=f"psA{b}_{tp}")
                    psB = pspool.tile([128, NT], f32, tag="ps", name=f"psB{b}_{tp}")
                    nA = tp * NT
                    nB = (tp + 1) * NT
                    for s in range(2):
                        for u in range(3):
                            off = s * W + u
                            first = s == 0 and u == 0
                            last = s == 1 and u == 2
                            nc.tensor.matmul(
                                psA[:], lhsT6[3 * s + u][:],
                                xs[:, nA + off : nA + off + NT],
                                start=first, stop=last,
                            )
                            nc.tensor.matmul(
                                psB[:], lhsT6[3 * s + u][:],
                                xs[:, nB + off : nB + off + NT],
                                start=first, stop=last,
                            )
                    # epilogue on DVE: scale+bias, both parities in one op
                    for n0, ps in ((nA, psA), (nB, psB)):
                        nc.vector.tensor_scalar(
                            out=outy[:, n0 : n0 + NT],
                            in0=ps[:],
                            scalar1=scale_vec[:],
                            scalar2=bias_vec[:],
                            op0=mult,
                            op1=add,
                        )
                    if b == 0 and tp == 2:
                        xs_list[1] = load_image(1)
                    if b == 0 and tp == 8:
                        xs_list[2] = load_image(2)
                    # stream out behind the epilogue: fix wrap cols, store.
                    # Steady images store in 2 groups; the last image per
                    # pair (8 row-pairs) to shrink the drain.
                    if b == BPC - 1:
                        bounds = [(tp, tp * 4, 8) for tp in range(0, NTILES, 2)]
                    else:
                        bounds = [(6, 0, 32), (12, 32, 24)]
                    if b > 0:
                        for tpb, r0, nr in bounds:
                            if tp != tpb:
                                continue
                            nc.vector.tensor_sub(
                                ove[:, 0, r0 : r0 + nr],
                                ove[:, 0, r0 : r0 + nr],
                                tmpL[:, r0 : r0 + nr],
                            )
                            nc.vector.tensor_sub(
                                ove[:, 111, r0 : r0 + nr],
                                ove[:, 111, r0 : r0 + nr],
                                tmpR[:, r0 : r0 + nr],
                            )
                            for pl, p0 in ((0, 0), (1, C)):
                                nc.scalar.dma_start(
                                    y_d[b, :, pl, r0 : r0 + nr, :],
                                    outy[
                                        p0 : p0 + C, r0 * W : (r0 + nr) * W
                                    ].rearrange("p (r w) -> p r w", w=W),
                                )
                if b == 0:
                    tmpL, tmpR = wrap_fixup(b, xs)
                    nc.vector.tensor_sub(ove[:, 0, :], ove[:, 0, :], tmpL[:])
                    nc.vector.tensor_sub(ove[:, 111, :], ove[:, 111, :], tmpR[:])
                    for r0 in range(0, HALF, 28):
                        for pl, p0 in ((0, 0), (1, C)):
                            nc.scalar.dma_start(
                                y_d[b, :, pl, r0 : r0 + 28, :],
                                outy[p0 : p0 + C, r0 * W : (r0 + 28) * W].rearrange(
                                    "p (r w) -> p r w", w=W
                                ),
                            )

    nc.compile()
    _CACHE["nc"] = nc
    return nc


def _run(inputs, trace=False):
    import ml_dtypes
    from concourse.bass_utils import run_bass_kernel_spmd

    nc = _build()
    # host-side: bf16 + row-parity-plane layout [B, C, 2, 56, W]
    x = (
        np.asarray(inputs["x"], dtype=np.float32)
        .astype(ml_dtypes.bfloat16)
        .reshape(B, C, HALF, 2, W)
        .transpose(0, 1, 3, 2, 4)
    )
    x = np.ascontiguousarray(x)
    shared = {
        "pweight": np.ascontiguousarray(
            np.asarray(inputs["pweight"], np.float32).astype(ml_dtypes.bfloat16)
        ),
        "nweight": np.ascontiguousarray(
            np.asarray(inputs["nweight"], np.float32).astype(ml_dtypes.bfloat16)
        ),
        "scale": np.ascontiguousarray(np.asarray(inputs["scale"], np.float32)),
        "pbias": np.ascontiguousarray(np.asarray(inputs["pbias"], np.float32)),
        "nbias": np.ascontiguousarray(np.asarray(inputs["nbias"], np.float32)),
        "biasscale": np.ascontiguousarray(np.asarray(inputs["biasscale"], np.float32)),
    }
    in_maps = [dict(shared, x=x[c * BPC : (c + 1) * BPC]) for c in range(CORES)]
    last_err = None
    for attempt in range(3):
        try:
            res = run_bass_kernel_spmd(
                nc, in_maps, core_ids=list(range(CORES)), trace=trace
            )
            y = np.concatenate(
                [np.asarray(res.results[c]["y"]) for c in range(CORES)], axis=0
            )
            # undo the parity-plane layout, upcast
            out = (
                y.reshape(B, C, 2, HALF, W)
                .transpose(0, 1, 3, 2, 4)
                .reshape(B, C, H, W)
                .astype(np.float32)
            )
            return np.ascontiguousarray(out), res.exec_time_ns
        except Exception as e:  # transient NRT_EXEC_UNIT_UNRECOVERABLE recovers on retry
            last_err = e
            import time

            time.sleep(10)
    raise last_err


def kernel(**inputs) -> np.ndarray:
    out, _ = _run(inputs)
    return out
